# revision 6
# baseline (speedup 1.0000x reference)
"""Trainium2 Bass kernel for nn_Decoder (LSTM decoder with residual output feedback).

Model (per batch row):
    h0 = c0 = z @ W_proj.T + b_proj                      # [B, H]
    y0 = x[:, -1, :]                                     # [B, X]
    per step t: gates = y_{t-1} @ W_ih.T + h_{t-1} @ W_hh.T + (b_ih + b_hh)
                i, f, g, o = split(gates); c = sig(f)*c + sig(i)*tanh(g)
                h = sig(o)*tanh(c); y_t = y_{t-1} + h @ W_out.T + b_out
    out = stack(y_1..y_T)                                # [B, T, Y]

Strategy: pure data-parallel over batch (B=1024 -> 128 rows/core on 8 cores,
weights replicated, zero collectives).  All state is kept TRANSPOSED on chip
([feature, batch] with batch as the 128-wide free dim): gates come out of the
PE array as gates^T with the weights stationary, and the elementwise state
update directly produces h^T, which feeds the next step's matmuls as the
moving operand -- no transposes anywhere in the recurrence.

Per-step structure (matmuls bf16, state fp32):
  - gate matmuls are ordered k-outer (contraction-chunk-outer) so each block
    only depends on one h^T chunk of the previous step; the y->gates matmuls
    (k=4, the W_ih path) run as a separate late block since they depend on
    y_{t-1} which is only ready after the previous step's h.
  - y accumulates in a persistent PSUM bank across all steps
    (y_t = y0 + psum after step t); the output is staged transposed in SBUF
    and the host untransposes the [64, T*128] result.
  - gate rows are pre-permuted so PSUM bank j holds [i_j | f_j | o_j | g_j];
    sigmoid(i,f,o) is one strided ACT instruction per bank pair.
  - elementwise work is split across DVE (c update, h, y adds) and GpSimd
    (sig(i)*tanh(g)), which have separate SBUF ports for these op shapes.
"""

import os
from contextlib import ExitStack

import ml_dtypes
import numpy as np

import concourse.bass as bass
import concourse.tile as tile
from concourse import bacc, mybir
from concourse.bass_utils import run_bass_kernel_spmd

F32 = mybir.dt.float32
BF16 = mybir.dt.bfloat16
SIG = mybir.ActivationFunctionType.Sigmoid
TANH = mybir.ActivationFunctionType.Tanh

B_TOT = 1024
N_CORES = 8
B = B_TOT // N_CORES  # 128 rows per core
ZD = 128
XD = 64
YD = 64
H = 512
GD = 4 * H  # 2048

HC = H // 128       # 4 H-chunks
GC = GD // 128      # 16 gate chunks

# Results of the last device run (exec_time_ns etc), for the test harness.
LAST_RESULTS = None

_BUILD_CACHE = {}


def _gate_row_order():
    """Permuted gate-row order: for H-chunk j, the 128-row blocks (i_j, f_j, o_j, g_j).

    Original gate layout along 4H: i=[0,512), f=[512,1024), g=[1024,1536), o=[1536,2048).
    """
    idx = []
    for j in range(HC):
        for base in (0, 512, 1536, 1024):  # i, f, o, g
            idx.extend(range(base + j * 128, base + (j + 1) * 128))
    return np.asarray(idx)


def _r3(ap, c):
    return ap.rearrange("p (u c) -> p u c", c=c)


def _build(T):
    ELEM_ENG_NAME = os.environ.get('T1_ENGINE', 'gpsimd')
    """Build the per-core Bass graph (identical on all cores)."""
    nc = bacc.Bacc("TRN2", target_bir_lowering=False, debug=False)

    # --- DRAM I/O (per-core shard layouts prepared on the host) ---
    d_zT = nc.dram_tensor("zT", [ZD, B], BF16, kind="ExternalInput")
    d_y0T = nc.dram_tensor("y0T", [YD, B], F32, kind="ExternalInput")
    d_wg_h = nc.dram_tensor("wg_h", [128, HC * GC * 128], BF16, kind="ExternalInput")
    d_wg_y = nc.dram_tensor("wg_y", [YD + 1, GC * 128], BF16, kind="ExternalInput")
    d_wout = nc.dram_tensor("wout", [128, HC * YD], BF16, kind="ExternalInput")
    d_bout1 = nc.dram_tensor("bout1", [1, YD], BF16, kind="ExternalInput")
    d_ones1 = nc.dram_tensor("ones1", [1, B], BF16, kind="ExternalInput")
    d_wproj = nc.dram_tensor("wproj", [ZD, H], BF16, kind="ExternalInput")
    d_bprojT = nc.dram_tensor("bprojT", [128, HC], F32, kind="ExternalInput")
    # output, transposed: out[y, t*128 + b] = y_t[b, y]; host untransposes
    d_out = nc.dram_tensor("out", [YD, T * B], F32, kind="ExternalOutput")

    with ExitStack() as ctx:
        tc = ctx.enter_context(tile.TileContext(nc))
        const = ctx.enter_context(tc.tile_pool(name="const", bufs=1))
        state = ctx.enter_context(tc.tile_pool(name="state", bufs=1))
        actp = ctx.enter_context(tc.tile_pool(name="actp", bufs=3))
        gpsum = ctx.enter_context(tc.tile_pool(name="gpsum", bufs=3, space="PSUM"))
        ypsum = ctx.enter_context(tc.tile_pool(name="ypsum", bufs=1, space="PSUM"))

        # --- constants in SBUF ---
        wg_h = const.tile([128, HC * GC * 128], BF16)
        wg_y = const.tile([YD + 1, GC * 128], BF16)
        wout = const.tile([128, HC * YD], BF16)
        bout1 = const.tile([1, YD], BF16)
        ones1 = const.tile([1, B], BF16)
        wproj = const.tile([ZD, H], BF16)
        bprojT = const.tile([128, HC], F32)
        zT = const.tile([ZD, B], BF16)
        y0T = const.tile([YD, B], F32)
        for sb, dr in (
            (wg_h, d_wg_h), (wg_y, d_wg_y), (wout, d_wout), (bout1, d_bout1),
            (ones1, d_ones1), (wproj, d_wproj), (bprojT, d_bprojT),
            (zT, d_zT), (y0T, d_y0T),
        ):
            nc.sync.dma_start(sb[:, :], dr[:, :])

        # --- persistent state (split per bank-pair so deps stay fine-grained) ---
        # cT[jj]: c^T for H-chunks (2jj, 2jj+1); hT[p][jj]: h^T double-buffered
        cT = [state.tile([128, 256], F32, name=f"cT{jj}") for jj in range(2)]
        hT = [[state.tile([128, 256], BF16, name=f"hT{p}_{jj}") for jj in range(2)]
              for p in range(2)]
        yTa = [state.tile([YD + 1, B], BF16, name=f"yTa{p}") for p in range(2)]
        ysbT = state.tile([YD, T * B], F32)     # output staging, transposed
        yp = ypsum.tile([YD, B], F32)           # persistent y-delta accumulator

        # --- init: h0 = c0 = z @ Wproj.T + bproj (transposed layout) ---
        h0p = gpsum.tile([128, 1024], F32, tag="g")
        for m in range(HC):
            nc.tensor.matmul(
                h0p[:, m * 128:(m + 1) * 128],
                lhsT=wproj[:, m * 128:(m + 1) * 128],
                rhs=zT[:, :],
                start=True, stop=True,
            )
        for m in range(HC):
            nc.vector.tensor_scalar_add(
                cT[m // 2][:, (m % 2) * 128:(m % 2) * 128 + 128],
                h0p[:, m * 128:(m + 1) * 128],
                bprojT[:, m:m + 1],
            )
        for jj in range(2):
            nc.vector.tensor_copy(hT[1][jj][:, :], cT[jj][:, :])  # cast to bf16

        # yTa holds [bf16(y); ones-row]; step t reads yTa[(t-1) % 2]
        nc.vector.tensor_copy(yTa[1][0:YD, :], y0T[:, :])
        nc.vector.memset(yTa[0][YD:YD + 1, :], 1.0)
        nc.vector.memset(yTa[1][YD:YD + 1, :], 1.0)

        DMA_CHUNK = 16

        def emit_y_tail(t):
            """y-path for step t: delta matmuls ran inside step t+1's k-blocks;
            here: bias accumulate + stage y_t (= y0 + psum) + bf16 copy for gates."""
            nc.tensor.matmul(yp[:, :], lhsT=bout1[0:1, :], rhs=ones1[0:1, :],
                             start=False, stop=(t == T - 1), skip_group_check=True)
            sl = ysbT[:, t * B:(t + 1) * B]
            nc.vector.tensor_add(sl, y0T[:, :], yp[:, :])
            if t < T - 1:
                nc.vector.tensor_add(yTa[t % 2][0:YD, :], y0T[:, :], yp[:, :])
            if t % DMA_CHUNK == DMA_CHUNK - 1 or t == T - 1:
                lo = (t // DMA_CHUNK) * DMA_CHUNK * B
                nc.sync.dma_start(d_out[:, lo:(t + 1) * B], ysbT[:, lo:(t + 1) * B])

        # --- the recurrence, fully unrolled ---
        for t in range(T):
            pv = (t + 1) % 2   # buffer holding state after step t-1
            cu = t % 2         # buffer written by step t
            gp = [gpsum.tile([128, 1024], F32, tag="g", name=f"gp{t}_{jj}")
                  for jj in range(2)]

            # gate matmuls, contraction-chunk-outer; y-delta matmuls of step
            # t-1 interleaved (same rhs dependency)
            for k in range(HC):
                if t > 0:
                    nc.tensor.matmul(
                        yp[:, :],
                        lhsT=wout[:, k * YD:(k + 1) * YD],
                        rhs=hT[pv][k // 2][:, (k % 2) * 128:(k % 2) * 128 + 128],
                        start=(t == 1 and k == 0), stop=False,
                        skip_group_check=True,
                    )
                for s in range(GC):
                    jj, gi = s // 8, s % 8
                    nc.tensor.matmul(
                        gp[jj][:, gi * 128:(gi + 1) * 128],
                        lhsT=wg_h[:, (k * GC + s) * 128:(k * GC + s + 1) * 128],
                        rhs=hT[pv][k // 2][:, (k % 2) * 128:(k % 2) * 128 + 128],
                        start=(k == 0), stop=False,
                        skip_group_check=True,
                    )
            if t > 0:
                emit_y_tail(t - 1)
            for s in range(GC):  # the W_ih / bias path, needs y_{t-1}
                jj, gi = s // 8, s % 8
                nc.tensor.matmul(
                    gp[jj][:, gi * 128:(gi + 1) * 128],
                    lhsT=wg_y[:, s * 128:(s + 1) * 128],
                    rhs=yTa[pv][:, :],
                    start=False, stop=True,
                    skip_group_check=True,
                )

            # activations + state update, per bank pair
            sg = [None, None]
            tg = [None, None]
            for jj in range(2):
                gp3 = _r3(gp[jj], 512)
                sg[jj] = actp.tile([128, 768], F32, tag=f"sg{jj}", name=f"sg{t}_{jj}")
                tg[jj] = actp.tile([128, 256], F32, tag=f"tg{jj}", name=f"tg{t}_{jj}")
                nc.scalar.activation(_r3(sg[jj], 384), gp3[:, :, 0:384], SIG)
                nc.scalar.activation(_r3(tg[jj], 128), gp3[:, :, 384:512], TANH)
            tch = [None, None]
            for jj in range(2):
                sg3 = _r3(sg[jj], 384)
                cs3 = _r3(cT[jj], 128)
                # t2 = sig(f)*c  (DVE);  t1 = sig(i)*tanh(g)  (GpSimd);  c = t1+t2
                t2 = actp.tile([128, 256], F32, tag=f"t2{jj}", name=f"t2_{t}_{jj}")
                nc.vector.tensor_mul(_r3(t2, 128), sg3[:, :, 128:256], cs3)
                t1 = actp.tile([128, 256], F32, tag=f"t1{jj}", name=f"t1_{t}_{jj}")
                (nc.gpsimd.tensor_mul if ELEM_ENG_NAME=='gpsimd' else nc.vector.tensor_mul)(_r3(t1, 128), sg3[:, :, 0:128], _r3(tg[jj], 128))
                nc.vector.tensor_add(cs3, _r3(t2, 128), _r3(t1, 128))
                tch[jj] = actp.tile([128, 256], F32, tag=f"tc{jj}", name=f"tc{t}_{jj}")
                nc.scalar.activation(_r3(tch[jj], 128), cs3, TANH)
            for jj in range(2):
                sg3 = _r3(sg[jj], 384)
                nc.vector.tensor_mul(_r3(hT[cu][jj], 128), sg3[:, :, 256:384],
                                     _r3(tch[jj], 128))

        # y-path tail for the final step
        for k in range(HC):
            nc.tensor.matmul(
                yp[:, :],
                lhsT=wout[:, k * YD:(k + 1) * YD],
                rhs=hT[(T - 1) % 2][k // 2][:, (k % 2) * 128:(k % 2) * 128 + 128],
                start=False, stop=False, skip_group_check=True,
            )
        emit_y_tail(T - 1)

    nc.compile()
    return nc


def _prep_consts(W_ih, W_hh, b_ih, b_hh, W_proj, b_proj, W_out, b_out):
    bf = ml_dtypes.bfloat16
    order = _gate_row_order()
    Wg = W_hh[order]                       # [2048, 512] permuted rows
    Wi = W_ih[order]                       # [2048, 64]
    bt = (b_ih + b_hh)[order]              # [2048]

    wg_h = np.empty((128, HC * GC * 128), dtype=bf)
    for k in range(HC):
        for s in range(GC):
            blk = Wg[s * 128:(s + 1) * 128, k * 128:(k + 1) * 128].T  # [K,M]
            wg_h[:, (k * GC + s) * 128:(k * GC + s + 1) * 128] = blk.astype(bf)

    wg_y = np.empty((YD + 1, GC * 128), dtype=bf)
    for s in range(GC):
        wg_y[0:YD, s * 128:(s + 1) * 128] = Wi[s * 128:(s + 1) * 128, :].T.astype(bf)
        wg_y[YD, s * 128:(s + 1) * 128] = bt[s * 128:(s + 1) * 128].astype(bf)

    wout = np.empty((128, HC * YD), dtype=bf)
    for k in range(HC):
        wout[:, k * YD:(k + 1) * YD] = W_out[:, k * 128:(k + 1) * 128].T.astype(bf)

    wproj = np.empty((ZD, H), dtype=bf)
    for m in range(HC):
        wproj[:, m * 128:(m + 1) * 128] = W_proj[m * 128:(m + 1) * 128, :].T.astype(bf)

    bprojT = b_proj.reshape(HC, 128).T.copy().astype(np.float32)  # [128, HC]
    bout1 = b_out.reshape(1, YD).astype(bf)
    ones1 = np.ones((1, B), dtype=bf)
    return dict(wg_h=wg_h, wg_y=wg_y, wout=wout, wproj=wproj,
                bprojT=bprojT, bout1=bout1, ones1=ones1)


def kernel(z, x, W_ih, W_hh, b_ih, b_hh, W_proj, b_proj, W_out, b_out, y_pred_len):
    global LAST_RESULTS
    z = np.asarray(z, dtype=np.float32)
    x = np.asarray(x, dtype=np.float32)
    T = int(np.asarray(y_pred_len))

    consts = _prep_consts(
        np.asarray(W_ih, np.float32), np.asarray(W_hh, np.float32),
        np.asarray(b_ih, np.float32), np.asarray(b_hh, np.float32),
        np.asarray(W_proj, np.float32), np.asarray(b_proj, np.float32),
        np.asarray(W_out, np.float32), np.asarray(b_out, np.float32),
    )

    if T not in _BUILD_CACHE:
        _BUILD_CACHE[T] = _build(T)
    nc = _BUILD_CACHE[T]

    bf = ml_dtypes.bfloat16
    in_maps = []
    for i in range(N_CORES):
        sl = slice(i * B, (i + 1) * B)
        m = dict(consts)
        m["zT"] = np.ascontiguousarray(z[sl].T.astype(bf))
        m["y0T"] = np.ascontiguousarray(x[sl, -1, :].T.astype(np.float32))
        in_maps.append(m)

    trace = bool(int(os.environ.get("BASS_KERNEL_TRACE", "0")))
    res = run_bass_kernel_spmd(
        nc, in_maps, core_ids=list(range(N_CORES)), trace=trace,
    )
    LAST_RESULTS = res

    outs = [np.ascontiguousarray(
                np.asarray(res.results[i]["out"]).reshape(YD, T, B).transpose(2, 1, 0))
            for i in range(N_CORES)]
    return np.concatenate(outs, axis=0)
